# revision 24
# baseline (speedup 1.0000x reference)
"""Trainium2 Bass kernel for the DMP-rollout Net (nn_Net_60567628808344).

Math
----
The reference integrates, per row r of p = (x*scale).reshape(-1, 27):
    y0 = p[:,0], goal = p[:,1], w = p[:,2:]
The whole 301-step rollout collapses to a closed form (constants built
exactly on host in float64):
    out[r, t] = A[t]*y0_r + B[t]*goal_r + (goal_r - y0_r) * (w_r @ H[t, :])
i.e. out = V @ HC with V (rows, 27) data, HC (27, 301) constant.

Device schedule (8-way batch data-parallel, bf16 in/out -- rel-err gate is
2e-2, bf16 end-to-end measures 7.3e-3):

WEIGHT-STATIONARY ORIENTATION: out^T[t, r] = HC^T V^T. The HC chunk
(<=128 t-cols) is loaded once per chunk via ldweights; V^T streams through
in 512-row groups as the MOVING operand. Consecutive same-weight matmuls
pipeline at ~N cycles each (drain overlaps next fill) -- no per-tile
LDWEIGHTS drain, unlike the data-stationary orientation. K is padded
27->32 and REPLICATED 4x to 128 rows (HC scaled by 1/4 -- exact in bf16):
the PE HAM clock-gate watches array-row activity, and only full-K
streaming sustains the busy window that ramps the PE 1.2 -> 2.4 GHz
(measured: K=32 streams stay at 427ns/512cols forever; K=128 reaches
217ns after the ~3.4us HAM window).

PSUM evacuation (the real wall: one PSUM read port per engine, fp32
source, 1 elem/cycle/lane) alternates DVE/ACT per group, weighted ~17:15
to the measured copy costs, so both engines drain concurrently. Each
output DMA must carry a single sem wait (walrus limit), so copies land in
engine-owned halves of the staging buffer; the output leaves the device
time-major and column-permuted, and the host undoes both during unshard.
"""

import numpy as np

# DMP hyperparameters fixed by Net.__init__ (hardcoded per problem spec)
N = 25
DOF = 2
DT = 0.01
TAU = 3.0
A_X = 2.0
A_Z = 48.0
B_Z = A_Z / 4.0
T = 301                    # time steps
BATCH = 65536
PARAM_DIM = DOF * (N + 2)  # 54
NCORES = 8

ROWS = BATCH * DOF         # 131072 (B*DOF rows)
RPC = ROWS // NCORES       # 16384 rows per core
RB = 512                   # streamed rows per matmul (one PSUM bank of fp32)
NG = RPC // RB             # 32 groups per chunk
CHUNKS = ((0, 128), (128, 128), (256, 45))   # (t0, height): 301 = 128+128+45
HEAD_GROUPS = 8            # V groups in the fast HWDGE input chunk
# The remaining 24 groups arrive on SWDGE in 4 pieces, each absorbed onto
# the PE clock by a tiny claim matmul just before its first consumer, so
# chunk-0 streaming overlaps the input transfer instead of stalling on one
# monolithic B-chunk wait (measured 10.7us dead time).
VB_PIECES = ((8, 14), (14, 20), (20, 26), (26, 32))  # [g0, g1) group ranges

# Copy engine per within-chunk group: 0=DVE, 1=ACT, weighted 17:15 to the
# measured FD=512 PSUM-copy costs (DVE ~683ns, ACT ~768ns).
ENG = [0 if (g % 2 == 0 or g == 31) else 1 for g in range(NG)]
ND = ENG.count(0)          # DVE groups own stage columns [0, 512*ND)
_dr = [g for g in range(NG) if ENG[g] == 0]
_ar = [g for g in range(NG) if ENG[g] == 1]
GOFF = {}                  # group -> stage column offset (engine halves)
for _i, _g in enumerate(_dr):
    GOFF[_g] = RB * _i
for _i, _g in enumerate(_ar):
    GOFF[_g] = RB * ND + RB * _i
LAST_D = max(_dr)
LAST_A = max(_ar)
DSPLIT = RB * (ND // 2 + 1)   # early first-chunk D DMA boundary
DSPLIT_G = _dr[ND // 2]       # ...emitted after this group

USE_BF16 = True


# ----------------------------------------------------------------------------
# Host-side constant build (exact, float64)
# ----------------------------------------------------------------------------
_const_cache = {}


def _build_constants(c=None, sigma2=None):
    """Return hc (32, 301) float32: row 0 -> A, 1 -> B, 2+n -> H[:, n],
    rows 27..31 zero."""
    if c is None:
        c = np.exp(-A_X * np.linspace(0.0, 1.0, N))
    if sigma2 is None:
        sigma2 = (N ** 1.5) / c / A_X
    c = np.asarray(c, np.float64)
    sigma2 = np.asarray(sigma2, np.float64)
    key = (c.tobytes(), sigma2.tobytes())
    if key in _const_cache:
        return _const_cache[key]

    k = DT / TAU
    M = np.array([[1.0, k], [-A_Z * B_Z * k, 1.0 - A_Z * k]])
    P = np.zeros(T + 1)
    Q = np.zeros(T + 1)
    Mn = np.eye(2)
    for n in range(T + 1):
        P[n] = Mn[0, 0]
        Q[n] = Mn[0, 1]
        Mn = Mn @ M

    decay = 1.0 - A_X * DT / TAU
    cx = decay ** np.arange(1, T + 1)                        # cx_1..cx_T
    psi = np.exp(-0.5 * (cx[:, None] - c[None, :]) ** 2 / sigma2[None, :])
    g = psi * (cx / psi.sum(1))[:, None]                     # (T, N)

    A = P[1:T + 1]
    B = k * A_Z * B_Z * np.cumsum(Q[0:T])
    # H[i] = k * sum_{m<=i} Q[i-m] g[m]  -- lower-triangular Toeplitz matvec
    ii = np.arange(T)[:, None]
    mm = np.arange(T)[None, :]
    L = np.where(ii >= mm, Q[np.clip(ii - mm, 0, T)], 0.0)   # (T, T)
    H = k * (L @ g)                                          # (T, N)

    hfull = np.zeros((32, T), np.float32)
    hfull[0] = A.astype(np.float32)
    hfull[1] = B.astype(np.float32)
    hfull[2:2 + N] = H.T.astype(np.float32)
    _const_cache[key] = hfull
    return hfull


def _pack_inputs(x, c, sigma2, scale):
    """Per-core va ([128, HEAD_GROUPS*RB + T]) and vb ([128, rest]) bf16."""
    x = np.asarray(x, np.float32)
    if scale is None:
        scale = np.ones(PARAM_DIM, np.float32)
    p = (x * np.asarray(scale, np.float32)).reshape(ROWS, N + 2)
    y0 = p[:, 0]
    goal = p[:, 1]
    u = goal - y0
    v = np.empty((ROWS, N + 2), np.float32)
    v[:, 0] = y0
    v[:, 1] = goal
    v[:, 2:] = p[:, 2:] * u[:, None]

    hc = _build_constants(c, sigma2)          # (32, 301) fp32
    # K replicated 4x (128 rows) with HC/4: exact in bf16, and the full-K
    # array activity is what lets the PE HAM clock-gate reach 2.4 GHz.
    hrep = np.tile(hc / 4.0, (4, 1))

    if USE_BF16:
        from ml_dtypes import bfloat16
        io_dt = bfloat16
    else:
        io_dt = np.float32

    head = HEAD_GROUPS * RB
    vas, vbs = [], []
    for i in range(NCORES):
        vc = v[RPC * i:RPC * (i + 1)]                 # (16384, 27)
        vt = np.zeros((32, RPC), np.float32)
        vt[:N + 2] = vc.T
        vt4 = np.tile(vt, (4, 1))                     # (128, 16384)
        va = np.concatenate([vt4[:, :head], hrep], axis=1)
        vas.append(np.ascontiguousarray(va.astype(io_dt)))
        vbs.append(np.ascontiguousarray(vt4[:, head:].astype(io_dt)))
    return vas, vbs


def _build_rhat():
    """Stage-column order -> true local row, for the host-side unshard."""
    rhat = np.empty(RPC, np.int64)
    for g in range(NG):
        rhat[RB * g:RB * (g + 1)] = GOFF[g] + np.arange(RB)
    return rhat


RHAT = _build_rhat()


# ----------------------------------------------------------------------------
# Bass kernel
# ----------------------------------------------------------------------------
_nc_cache = []


def _build_bass():
    if _nc_cache:
        return _nc_cache[0]
    import concourse.bass as bass
    import concourse.mybir as mybir
    from concourse import tile
    import bass_rust
    from concourse.vector_clock import ScopedClock

    class SplitDrainTileContext(tile.TileContext):
        """This walrus build allows a single sync wait per instruction, but
        TileContext's kernel-tail drain carries one wait per live sem lane.
        Split the extras onto standalone single-wait SP nops (same stream, so
        all waits still complete before the barrier + sem clearing)."""

        def _drain_and_barrier(self, tick_clock, wait_clock):
            nc = self.nc
            drain_inst = nc.sync.drain()
            wait_clock.add_sem_waits(
                drain_inst.ins, ScopedClock({None: tick_clock.global_clock})
            )
            si = drain_inst.ins.sync_info
            waits = list(si.on_wait) if si is not None else []
            if len(waits) > 1:
                drain_inst.ins.sync_info = bass_rust.SyncInfo(
                    on_wait=[waits[0]], on_update=list(si.on_update)
                )
                for w in waits[1:]:
                    n = nc.sync.nop(nofuse=True)
                    n.ins.sync_info = bass_rust.SyncInfo(
                        on_wait=[w], on_update=[]
                    )
            nc.all_engine_barrier()
            assert self.sems is not None
            popped = nc._tile_sem_poison_stack.pop()
            assert popped is self._sem_poison
            nc.clear_and_free_semaphores(list(self.sems.allocated().values()))
            nc.all_engine_barrier()

    f32 = mybir.dt.float32
    fio = mybir.dt.bfloat16 if USE_BF16 else f32
    nc = bass.Bass()
    head = HEAD_GROUPS * RB
    va_d = nc.dram_tensor("va", [128, head + T], fio, kind="ExternalInput")
    vb_d = nc.dram_tensor("vb", [128, RPC - head], fio, kind="ExternalInput")
    # output leaves TRANSPOSED (time-major) + column-permuted; host undoes
    out_d = nc.dram_tensor("out", [T, RPC], fio, kind="ExternalOutput")

    pe = nc.tensor

    def mm_stream(out_sl, rhs_sl, w_sl, h):
        """Streaming matmul against resident weights: same weights across a
        chunk's 32 groups, so back-to-back matmuls cost ~N cycles each."""
        csz = 32 if h <= 32 else (64 if h <= 64 else 128)
        ifmap_ap = pe.lower_ap(rhs_sl.opt({0}), opt=False)
        weights_ap = pe.lower_ap(
            w_sl.opt({0}), opt=False, for_matmul_weights=True
        )
        out_ap = pe.lower_ap(out_sl)
        return pe.add_instruction(
            mybir.InstMatmult(
                name=pe.bass.get_next_instruction_name(),
                replication_resolution=0,
                replication_shift_amnt=0,
                replication_num_rows=0,
                start_tensor_calc=True,
                stop_tensor_calc=True,
                ins=[ifmap_ap, weights_ap],
                outs=[out_ap],
                tile_position=(0, 0),
                tile_size=(128, csz),
                ldweights=True,
            )
        )

    with SplitDrainTileContext(nc) as tc:
        with (
            tc.tile_pool(name="vtp", bufs=1) as vtp,
            tc.tile_pool(name="stage", bufs=1) as stagep,
            tc.tile_pool(name="psum", bufs=7, space="PSUM") as psump,
            tc.tile_pool(name="clm", bufs=1, space="PSUM") as clmp,
        ):
            vtsA = vtp.tile([128, head + T], fio, tag="vtsA")
            vtsB = vtp.tile([128, RPC - head], fio, tag="vtsB")
            nc.sync.dma_start(vtsA[:], va_d[:])
            for g0, g1 in VB_PIECES:
                b0, b1 = RB * g0 - head, RB * g1 - head
                nc.gpsimd.dma_start(vtsB[:, b0:b1], vb_d[:, b0:b1])
            hsb = vtsA[:, head:head + T]

            def vgrp(g):
                if g < HEAD_GROUPS:
                    return vtsA[:, RB * g:RB * (g + 1)]
                gb = g - HEAD_GROUPS
                return vtsB[:, RB * gb:RB * (gb + 1)]

            # One persistent staging buffer; no slot recycling -> no release
            # waits on copies (single sync wait per instruction).
            stage = stagep.tile([128, len(CHUNKS), RPC], fio)

            for ci, (t0, h) in enumerate(CHUNKS):
                nc.tensor.ldweights(hsb[:, t0:t0 + h])
                for g in range(NG):
                    if ci == 0 and g in {p[0] for p in VB_PIECES}:
                        # Tiny claim matmul: absorbs this vb piece's DMA wait
                        # on the PE clock so the piece's streams carry only
                        # their psum-release wait.
                        b0 = RB * g - head
                        cps = clmp.tile([128, 8], f32)
                        nc.tensor.matmul(
                            cps[:1, :1],
                            vtsB[:1, b0:b0 + 1],
                            vtsB[:1, b0 + 1:b0 + 2],
                            start=True,
                            stop=True,
                            tile_position=(0, 0),
                        )
                    ps = psump.tile([128, RB], f32)
                    mm_stream(ps[0:h, :], vgrp(g), hsb[:, t0:t0 + h], h)
                    dst = stage[0:h, ci, GOFF[g]:GOFF[g] + RB]
                    if ENG[g] == 0:
                        nc.vector.tensor_copy(dst, ps[0:h, :])
                    else:
                        nc.scalar.copy(dst, ps[0:h, :])
                    # Writeback DMAs: each covers one engine's contiguous
                    # stage half (single sem wait); 6 half-chunk DMAs + one
                    # early first-chunk split = 7 (input takes the 8th lane).
                    if ci == 0 and g == DSPLIT_G:
                        nc.sync.dma_start(
                            out_d[t0:t0 + h, 0:DSPLIT],
                            stage[0:h, ci, 0:DSPLIT],
                        )
                    elif g == LAST_A:
                        nc.sync.dma_start(
                            out_d[t0:t0 + h, RB * ND:RPC],
                            stage[0:h, ci, RB * ND:RPC],
                        )
                    elif g == LAST_D:
                        lo = DSPLIT if ci == 0 else 0
                        nc.sync.dma_start(
                            out_d[t0:t0 + h, lo:RB * ND],
                            stage[0:h, ci, lo:RB * ND],
                        )

    _nc_cache.append(nc)
    return nc


def _run(in_maps, trace=False):
    from concourse.bass_utils import run_bass_kernel_spmd

    nc = _build_bass()
    return run_bass_kernel_spmd(nc, in_maps, list(range(NCORES)), trace=trace)


def kernel(x, c=None, sigma2=None, scale=None, _trace=False):
    vas, vbs = _pack_inputs(x, c, sigma2, scale)
    in_maps = [{"va": vas[i], "vb": vbs[i]} for i in range(NCORES)]
    res = _run(in_maps, trace=_trace)
    outs = []
    for i in range(NCORES):
        arr = np.asarray(res.results[i]["out"], np.float32)  # [T, RPC] staged
        outs.append(arr.T[RHAT])                             # [RPC, T] true r
    out = np.concatenate(outs, axis=0).reshape(BATCH, DOF, T)
    if _trace:
        return out, res
    return out


# revision 25
# speedup vs baseline: 1.0127x; 1.0127x over previous
"""Trainium2 Bass kernel for the DMP-rollout Net (nn_Net_60567628808344).

Math
----
The reference integrates, per row r of p = (x*scale).reshape(-1, 27):
    y0 = p[:,0], goal = p[:,1], w = p[:,2:]
The whole 301-step rollout collapses to a closed form (constants built
exactly on host in float64):
    out[r, t] = A[t]*y0_r + B[t]*goal_r + (goal_r - y0_r) * (w_r @ H[t, :])
i.e. out = V @ HC with V (rows, 27) data, HC (27, 301) constant.

Device schedule (8-way batch data-parallel, bf16 in/out -- rel-err gate is
2e-2, bf16 end-to-end measures 7.3e-3):

WEIGHT-STATIONARY ORIENTATION: out^T[t, r] = HC^T V^T. The HC chunk
(<=128 t-cols) is loaded once per chunk via ldweights; V^T streams through
in 512-row groups as the MOVING operand. Consecutive same-weight matmuls
pipeline at ~N cycles each (drain overlaps next fill) -- no per-tile
LDWEIGHTS drain, unlike the data-stationary orientation. K is padded
27->32 and REPLICATED 4x to 128 rows (HC scaled by 1/4 -- exact in bf16):
the PE HAM clock-gate watches array-row activity, and only full-K
streaming sustains the busy window that ramps the PE 1.2 -> 2.4 GHz
(measured: K=32 streams stay at 427ns/512cols forever; K=128 reaches
217ns after the ~3.4us HAM window).

PSUM evacuation (the real wall: one PSUM read port per engine, fp32
source, 1 elem/cycle/lane) alternates DVE/ACT per group, weighted ~17:15
to the measured copy costs, so both engines drain concurrently. Each
output DMA must carry a single sem wait (walrus limit), so copies land in
engine-owned halves of the staging buffer; the output leaves the device
time-major and column-permuted, and the host undoes both during unshard.
"""

import numpy as np

# DMP hyperparameters fixed by Net.__init__ (hardcoded per problem spec)
N = 25
DOF = 2
DT = 0.01
TAU = 3.0
A_X = 2.0
A_Z = 48.0
B_Z = A_Z / 4.0
T = 301                    # time steps
BATCH = 65536
PARAM_DIM = DOF * (N + 2)  # 54
NCORES = 8

ROWS = BATCH * DOF         # 131072 (B*DOF rows)
RPC = ROWS // NCORES       # 16384 rows per core
RB = 512                   # streamed rows per matmul (one PSUM bank of fp32)
NG = RPC // RB             # 32 groups per chunk
CHUNKS = ((0, 128), (128, 128), (256, 45))   # (t0, height): 301 = 128+128+45
# Input routing, from measured path characteristics: the sync-HWDGE input
# path is low-latency but only ~140 GB/s, while SWDGE spins up ~4us then
# sustains ~385 GB/s. So the sync ring carries just hc + 2 head groups
# (~0.3MB, ready in ~2us) and the remaining 30 groups arrive on SWDGE in
# 5 pieces, each absorbed onto the PE clock by a tiny claim matmul right
# before its first consumer, so chunk-0 streaming rides the incoming
# transfer instead of stalling on one monolithic wait.
HEAD_GROUPS = 2
VB_PIECES = ((2, 10), (10, 16), (16, 22), (22, 28), (28, 32))

# Copy engine per within-chunk group: 0=DVE, 1=ACT, weighted 17:15 to the
# measured FD=512 PSUM-copy costs (DVE ~683ns, ACT ~768ns).
ENG = [0 if (g % 2 == 0 or g == 31) else 1 for g in range(NG)]
ND = ENG.count(0)          # DVE groups own stage columns [0, 512*ND)
_dr = [g for g in range(NG) if ENG[g] == 0]
_ar = [g for g in range(NG) if ENG[g] == 1]
GOFF = {}                  # group -> stage column offset (engine halves)
for _i, _g in enumerate(_dr):
    GOFF[_g] = RB * _i
for _i, _g in enumerate(_ar):
    GOFF[_g] = RB * ND + RB * _i
LAST_D = max(_dr)
LAST_A = max(_ar)
DSPLIT = RB * (ND // 2 + 1)   # early first-chunk D DMA boundary
DSPLIT_G = _dr[ND // 2]       # ...emitted after this group

USE_BF16 = True


# ----------------------------------------------------------------------------
# Host-side constant build (exact, float64)
# ----------------------------------------------------------------------------
_const_cache = {}


def _build_constants(c=None, sigma2=None):
    """Return hc (32, 301) float32: row 0 -> A, 1 -> B, 2+n -> H[:, n],
    rows 27..31 zero."""
    if c is None:
        c = np.exp(-A_X * np.linspace(0.0, 1.0, N))
    if sigma2 is None:
        sigma2 = (N ** 1.5) / c / A_X
    c = np.asarray(c, np.float64)
    sigma2 = np.asarray(sigma2, np.float64)
    key = (c.tobytes(), sigma2.tobytes())
    if key in _const_cache:
        return _const_cache[key]

    k = DT / TAU
    M = np.array([[1.0, k], [-A_Z * B_Z * k, 1.0 - A_Z * k]])
    P = np.zeros(T + 1)
    Q = np.zeros(T + 1)
    Mn = np.eye(2)
    for n in range(T + 1):
        P[n] = Mn[0, 0]
        Q[n] = Mn[0, 1]
        Mn = Mn @ M

    decay = 1.0 - A_X * DT / TAU
    cx = decay ** np.arange(1, T + 1)                        # cx_1..cx_T
    psi = np.exp(-0.5 * (cx[:, None] - c[None, :]) ** 2 / sigma2[None, :])
    g = psi * (cx / psi.sum(1))[:, None]                     # (T, N)

    A = P[1:T + 1]
    B = k * A_Z * B_Z * np.cumsum(Q[0:T])
    # H[i] = k * sum_{m<=i} Q[i-m] g[m]  -- lower-triangular Toeplitz matvec
    ii = np.arange(T)[:, None]
    mm = np.arange(T)[None, :]
    L = np.where(ii >= mm, Q[np.clip(ii - mm, 0, T)], 0.0)   # (T, T)
    H = k * (L @ g)                                          # (T, N)

    hfull = np.zeros((32, T), np.float32)
    hfull[0] = A.astype(np.float32)
    hfull[1] = B.astype(np.float32)
    hfull[2:2 + N] = H.T.astype(np.float32)
    _const_cache[key] = hfull
    return hfull


def _pack_inputs(x, c, sigma2, scale):
    """Per-core va ([128, HEAD_GROUPS*RB + T]) and vb ([128, rest]) bf16."""
    x = np.asarray(x, np.float32)
    if scale is None:
        scale = np.ones(PARAM_DIM, np.float32)
    p = (x * np.asarray(scale, np.float32)).reshape(ROWS, N + 2)
    y0 = p[:, 0]
    goal = p[:, 1]
    u = goal - y0
    v = np.empty((ROWS, N + 2), np.float32)
    v[:, 0] = y0
    v[:, 1] = goal
    v[:, 2:] = p[:, 2:] * u[:, None]

    hc = _build_constants(c, sigma2)          # (32, 301) fp32
    # K replicated 4x (128 rows) with HC/4: exact in bf16, and the full-K
    # array activity is what lets the PE HAM clock-gate reach 2.4 GHz.
    hrep = np.tile(hc / 4.0, (4, 1))

    if USE_BF16:
        from ml_dtypes import bfloat16
        io_dt = bfloat16
    else:
        io_dt = np.float32

    head = HEAD_GROUPS * RB
    vas, vbs = [], []
    for i in range(NCORES):
        vc = v[RPC * i:RPC * (i + 1)]                 # (16384, 27)
        vt = np.zeros((32, RPC), np.float32)
        vt[:N + 2] = vc.T
        vt4 = np.tile(vt, (4, 1))                     # (128, 16384)
        va = np.concatenate([vt4[:, :head], hrep], axis=1)
        vas.append(np.ascontiguousarray(va.astype(io_dt)))
        vbs.append(np.ascontiguousarray(vt4[:, head:].astype(io_dt)))
    return vas, vbs


def _build_rhat():
    """Stage-column order -> true local row, for the host-side unshard."""
    rhat = np.empty(RPC, np.int64)
    for g in range(NG):
        rhat[RB * g:RB * (g + 1)] = GOFF[g] + np.arange(RB)
    return rhat


RHAT = _build_rhat()


# ----------------------------------------------------------------------------
# Bass kernel
# ----------------------------------------------------------------------------
_nc_cache = []


def _build_bass():
    if _nc_cache:
        return _nc_cache[0]
    import concourse.bass as bass
    import concourse.mybir as mybir
    from concourse import tile
    import bass_rust
    from concourse.vector_clock import ScopedClock

    class SplitDrainTileContext(tile.TileContext):
        """This walrus build allows a single sync wait per instruction, but
        TileContext's kernel-tail drain carries one wait per live sem lane.
        Split the extras onto standalone single-wait SP nops (same stream, so
        all waits still complete before the barrier + sem clearing)."""

        def _drain_and_barrier(self, tick_clock, wait_clock):
            nc = self.nc
            drain_inst = nc.sync.drain()
            wait_clock.add_sem_waits(
                drain_inst.ins, ScopedClock({None: tick_clock.global_clock})
            )
            si = drain_inst.ins.sync_info
            waits = list(si.on_wait) if si is not None else []
            if len(waits) > 1:
                drain_inst.ins.sync_info = bass_rust.SyncInfo(
                    on_wait=[waits[0]], on_update=list(si.on_update)
                )
                for w in waits[1:]:
                    n = nc.sync.nop(nofuse=True)
                    n.ins.sync_info = bass_rust.SyncInfo(
                        on_wait=[w], on_update=[]
                    )
            nc.all_engine_barrier()
            assert self.sems is not None
            popped = nc._tile_sem_poison_stack.pop()
            assert popped is self._sem_poison
            nc.clear_and_free_semaphores(list(self.sems.allocated().values()))
            nc.all_engine_barrier()

    f32 = mybir.dt.float32
    fio = mybir.dt.bfloat16 if USE_BF16 else f32
    nc = bass.Bass()
    head = HEAD_GROUPS * RB
    va_d = nc.dram_tensor("va", [128, head + T], fio, kind="ExternalInput")
    vb_d = nc.dram_tensor("vb", [128, RPC - head], fio, kind="ExternalInput")
    # output leaves TRANSPOSED (time-major) + column-permuted; host undoes
    out_d = nc.dram_tensor("out", [T, RPC], fio, kind="ExternalOutput")

    pe = nc.tensor

    def mm_stream(out_sl, rhs_sl, w_sl, h):
        """Streaming matmul against resident weights: same weights across a
        chunk's 32 groups, so back-to-back matmuls cost ~N cycles each."""
        csz = 32 if h <= 32 else (64 if h <= 64 else 128)
        ifmap_ap = pe.lower_ap(rhs_sl.opt({0}), opt=False)
        weights_ap = pe.lower_ap(
            w_sl.opt({0}), opt=False, for_matmul_weights=True
        )
        out_ap = pe.lower_ap(out_sl)
        return pe.add_instruction(
            mybir.InstMatmult(
                name=pe.bass.get_next_instruction_name(),
                replication_resolution=0,
                replication_shift_amnt=0,
                replication_num_rows=0,
                start_tensor_calc=True,
                stop_tensor_calc=True,
                ins=[ifmap_ap, weights_ap],
                outs=[out_ap],
                tile_position=(0, 0),
                tile_size=(128, csz),
                ldweights=True,
            )
        )

    with SplitDrainTileContext(nc) as tc:
        with (
            tc.tile_pool(name="vtp", bufs=1) as vtp,
            tc.tile_pool(name="stage", bufs=1) as stagep,
            tc.tile_pool(name="psum", bufs=7, space="PSUM") as psump,
            tc.tile_pool(name="clm", bufs=1, space="PSUM") as clmp,
        ):
            vtsA = vtp.tile([128, head + T], fio, tag="vtsA")
            vtsB = vtp.tile([128, RPC - head], fio, tag="vtsB")
            nc.sync.dma_start(vtsA[:], va_d[:])
            for g0, g1 in VB_PIECES:
                b0, b1 = RB * g0 - head, RB * g1 - head
                nc.gpsimd.dma_start(vtsB[:, b0:b1], vb_d[:, b0:b1])
            hsb = vtsA[:, head:head + T]

            def vgrp(g):
                if g < HEAD_GROUPS:
                    return vtsA[:, RB * g:RB * (g + 1)]
                gb = g - HEAD_GROUPS
                return vtsB[:, RB * gb:RB * (gb + 1)]

            # One persistent staging buffer; no slot recycling -> no release
            # waits on copies (single sync wait per instruction).
            stage = stagep.tile([128, len(CHUNKS), RPC], fio)

            for ci, (t0, h) in enumerate(CHUNKS):
                nc.tensor.ldweights(hsb[:, t0:t0 + h])
                for g in range(NG):
                    if ci == 0 and g in {p[0] for p in VB_PIECES}:
                        # Tiny claim matmul: absorbs this vb piece's DMA wait
                        # on the PE clock so the piece's streams carry only
                        # their psum-release wait.
                        b0 = RB * g - head
                        cps = clmp.tile([128, 8], f32)
                        nc.tensor.matmul(
                            cps[:1, :1],
                            vtsB[:1, b0:b0 + 1],
                            vtsB[:1, b0 + 1:b0 + 2],
                            start=True,
                            stop=True,
                            tile_position=(0, 0),
                        )
                    ps = psump.tile([128, RB], f32)
                    mm_stream(ps[0:h, :], vgrp(g), hsb[:, t0:t0 + h], h)
                    dst = stage[0:h, ci, GOFF[g]:GOFF[g] + RB]
                    if ENG[g] == 0:
                        nc.vector.tensor_copy(dst, ps[0:h, :])
                    else:
                        nc.scalar.copy(dst, ps[0:h, :])
                    # Writeback DMAs: each covers one engine's contiguous
                    # stage half (single sem wait); 6 half-chunk DMAs + one
                    # early first-chunk split = 7 (input takes the 8th lane).
                    if ci == 0 and g == DSPLIT_G:
                        nc.sync.dma_start(
                            out_d[t0:t0 + h, 0:DSPLIT],
                            stage[0:h, ci, 0:DSPLIT],
                        )
                    elif g == LAST_A:
                        nc.sync.dma_start(
                            out_d[t0:t0 + h, RB * ND:RPC],
                            stage[0:h, ci, RB * ND:RPC],
                        )
                    elif g == LAST_D:
                        lo = DSPLIT if ci == 0 else 0
                        nc.sync.dma_start(
                            out_d[t0:t0 + h, lo:RB * ND],
                            stage[0:h, ci, lo:RB * ND],
                        )

    _nc_cache.append(nc)
    return nc


def _run(in_maps, trace=False):
    from concourse.bass_utils import run_bass_kernel_spmd

    nc = _build_bass()
    return run_bass_kernel_spmd(nc, in_maps, list(range(NCORES)), trace=trace)


def kernel(x, c=None, sigma2=None, scale=None, _trace=False):
    vas, vbs = _pack_inputs(x, c, sigma2, scale)
    in_maps = [{"va": vas[i], "vb": vbs[i]} for i in range(NCORES)]
    res = _run(in_maps, trace=_trace)
    outs = []
    for i in range(NCORES):
        arr = np.asarray(res.results[i]["out"], np.float32)  # [T, RPC] staged
        outs.append(arr.T[RHAT])                             # [RPC, T] true r
    out = np.concatenate(outs, axis=0).reshape(BATCH, DOF, T)
    if _trace:
        return out, res
    return out
